# revision 59
# baseline (speedup 1.0000x reference)
"""GQA forward (b=2, s=2048, H=32 q heads, 8 kv heads, d=64) on 8 TRN2 cores.

Sharding: core k owns query heads 4k..4k+3 and kv head k. GQA group
structure makes attention fully local per core (q heads 4k..4k+3 attend
only to kv head k). x is replicated; W columns are sharded; outputs are
column-concatenated.

Per-core kernel (Tile framework), v2 — bf16 + transposed dataflow:
  - Host passes x.T (pre-transposed, bf16) so no PE transposes are spent
    producing x.T tiles; W is passed bf16 with head-dim columns
    de-interleaved (evens then odds per head) so RoPE works on
    contiguous partition blocks in the transposed projection layout.
  - Projections directly in transposed layout: QKV.T[cols,s] tiles =
    W_chunk.T @ xT_chunk accumulated over 16 k-chunks (W stationary).
    Col-blocks: [q0|q1], [q2|q3], [k|v], each row-packed [e32 o32] per
    head.
  - RoPE fused with PSUM eviction on DVE: 32-row partition-block ops,
    f32 intermediates, bf16 results straight into Q tiles / K resident.
  - V.T rows are flipped to natural [kv, d] via 4 small PE transposes
    per s-tile into the [V|1] resident.
  - Attention in transposed layout (bf16): S.T[kv,q] = K @ Q.T per
    128-kv block, exp on ACT (scale=1/8 folded); full (off-diagonal)
    blocks are computed in pairs sharing one 2-bank PSUM tile and one
    ACTIVATE (amortizes the ~352-cycle ACT ramp); causal via block
    skipping + triangular predicated masks on diagonal blocks;
    ctx.T[65,q] = [V|1].T @ P.T accumulated in PSUM (row 64 = sums).
  - Finalize in transposed layout: reciprocal of the sums row (DVE),
    partition_broadcast (GpSimd) to 64 rows, one TT multiply, DMA out
    transposed [256, B*S]; host transposes back. No PE involvement.
"""

import numpy as np
from contextlib import ExitStack

import ml_dtypes

import concourse.bass as bass
import concourse.bacc as bacc
import concourse.mybir as mybir
from concourse import tile
from concourse.bass_utils import run_bass_kernel_spmd

F32 = mybir.dt.float32
F32R = mybir.dt.float32r
BF16 = mybir.dt.bfloat16
U8 = mybir.dt.uint8
BF16NP = ml_dtypes.bfloat16
MUL = mybir.AluOpType.mult
ADD = mybir.AluOpType.add

B = 2
S = 2048
DIN = 2048
D = 64              # head dim
HPC = 4             # query heads per core
NCORES = 8
WCOLS = 384         # 3 col-blocks of 128: [q0|q1], [q2|q3], [k|v]
ST = 512            # s-tile (rows per outer step)
NST = B * S // ST   # 8 s-tiles
NCH = DIN // 128    # 16 k-chunks
NKV = S // 128      # kv tiles per batch
NEG = -30000.0      # pre-scale mask fill; exp(NEG/8) == 0 in f32


def build_bass():
    nc = bacc.Bacc(None, target_bir_lowering=False)
    xt_d = nc.declare_dram_parameter("xt", [DIN, B * S], BF16, isOutput=False)
    w_d = nc.declare_dram_parameter("w", [DIN, WCOLS], BF16, isOutput=False)
    cos_d = nc.declare_dram_parameter("cosq", [128, S], BF16, isOutput=False)
    sin_d = nc.declare_dram_parameter("sinq", [128, S], BF16, isOutput=False)
    mask_d = nc.declare_dram_parameter("mask", [128, 128], U8, isOutput=False)
    id_d = nc.declare_dram_parameter("ident", [128, 128], BF16, isOutput=False)
    out_d = nc.declare_dram_parameter("out", [B * S, HPC * D], F32, isOutput=True)

    with ExitStack() as ctx:
        tc = ctx.enter_context(tile.TileContext(nc))
        const = ctx.enter_context(tc.tile_pool(name="const", bufs=1))
        resid = ctx.enter_context(tc.tile_pool(name="resid", bufs=1))
        xa_p = ctx.enter_context(tc.tile_pool(name="xa", bufs=3))
        qt_p = ctx.enter_context(tc.tile_pool(name="qt", bufs=4))
        tmp_p = ctx.enter_context(tc.tile_pool(name="tmp", bufs=3))
        vt_p = ctx.enter_context(tc.tile_pool(name="vt", bufs=2))
        p_p = ctx.enter_context(tc.tile_pool(name="p", bufs=6))
        cx_p = ctx.enter_context(tc.tile_pool(name="cx", bufs=4))
        o_p = ctx.enter_context(tc.tile_pool(name="o", bufs=8))
        rv_p = ctx.enter_context(tc.tile_pool(name="rv", bufs=8))
        pr_ps = ctx.enter_context(tc.tile_pool(name="pr_ps", bufs=2, space="PSUM"))
        sc_ps = ctx.enter_context(tc.tile_pool(name="sc_ps", bufs=3, space="PSUM"))
        cx_ps = ctx.enter_context(tc.tile_pool(name="cx_ps", bufs=2, space="PSUM"))
        fi_ps = ctx.enter_context(tc.tile_pool(name="fi_ps", bufs=1, space="PSUM"))

        # constants — w split so the first matmul can start early (x.T
        # tiles stream in parallel on the scalar hwdge queue)
        w_dram = w_d.rearrange("(c p) n -> p c n", p=128)
        w_sb = const.tile([128, NCH, WCOLS], BF16)
        for c2 in range(0, NCH, 2):
            nc.sync.dma_start(out=w_sb[:, c2:c2 + 2, :],
                              in_=w_dram[:, c2:c2 + 2, :])
        cos_sb = const.tile([128, S], BF16)
        sin_sb = const.tile([128, S], BF16)
        # first s-tile's cos/sin lands early; later slices stream per s-tile
        nc.sync.dma_start(out=cos_sb[:, 0:ST], in_=cos_d[:, 0:ST])
        nc.sync.dma_start(out=sin_sb[:, 0:ST], in_=sin_d[:, 0:ST])
        mask_sb = const.tile([128, 128], U8)
        nc.sync.dma_start(out=mask_sb[:], in_=mask_d[:])
        ident = const.tile([128, 128], BF16)
        nc.sync.dma_start(out=ident[:], in_=id_d[:])
        neg_sb = const.tile([128, 128], F32)
        nc.vector.memset(neg_sb[:], NEG)

        # K.T resident (RoPE'd, bf16); rows 64-127 duplicate rows 0-63 so
        # the scores lhsT can match either base partition of the Q halves.
        kt_res = resid.tile([128, B * S], BF16)
        # [V | 1 | pad] kv-tiles, natural [kv, d] layout
        vp_res = resid.tile([128, B * NKV, 66], BF16)
        nc.vector.memset(vp_res[:, :, 64:65], 1.0)

        xt_dram = xt_d.rearrange("(c p) s -> p c s", p=128)

        # x.T tiles are prefetched one s-tile ahead on the scalar hwdge
        # queue (chunk-split so the first projection can start early)
        xa_tiles = {}

        def load_xa(st, step=4):
            xa = xa_p.tile([128, NCH, ST], BF16, tag="xa", name=f"xa{st}")
            for c4 in range(0, NCH, step):
                nc.scalar.dma_start(
                    out=xa[:, c4:c4 + step, :],
                    in_=xt_dram[:, c4:c4 + step, st * ST:(st + 1) * ST])
            xa_tiles[st] = xa

        load_xa(0, step=2)

        for st in range(NST):
            b, sti = divmod(st, 4)
            scol = slice(sti * ST, (sti + 1) * ST)  # within-batch position
            xa = xa_tiles.pop(st)
            if b == 0 and sti + 1 < 4:
                nxt = slice((sti + 1) * ST, (sti + 2) * ST)
                nc.scalar.dma_start(out=cos_sb[:, nxt], in_=cos_d[:, nxt])
                nc.scalar.dma_start(out=sin_sb[:, nxt], in_=sin_d[:, nxt])

            # ---- projections (transposed layout) + RoPE ----
            qa = qt_p.tile([128, ST], BF16, tag="qa")   # [q0_e q0_o q1_e q1_o]
            qb = qt_p.tile([128, ST], BF16, tag="qb")   # [q2_e q2_o q3_e q3_o]
            for cb in range(3):
                pp = pr_ps.tile([128, ST], F32, tag="pp")
                for c in range(NCH):
                    nc.tensor.matmul(
                        pp[:], w_sb[:, c, cb * 128:(cb + 1) * 128], xa[:, c, :],
                        start=(c == 0), stop=(c == NCH - 1))
                ts = tmp_p.tile([128, ST], F32, tag="ts")
                qn = tmp_p.tile([128, ST], F32, tag="qn")
                if cb < 2:
                    dst = qa if cb == 0 else qb
                    # rows per head h (0/1): [e at 64h..64h+32, o at +32..+64]
                    for hh in range(2):
                        r = 64 * hh
                        nc.vector.scalar_tensor_tensor(
                            ts[r:r + 32, :], pp[r + 32:r + 64, :], -1.0,
                            sin_sb[r:r + 32, scol], MUL, MUL)
                        nc.vector.tensor_tensor(
                            ts[r + 32:r + 64, :], pp[r:r + 32, :],
                            sin_sb[r + 32:r + 64, scol], MUL)
                    nc.vector.tensor_tensor(qn[:], pp[:], cos_sb[:, scol], MUL)
                    nc.vector.tensor_tensor(dst[:], qn[:], ts[:], ADD)
                else:
                    # rows: [k_e(32) k_o(32) | v(64)]
                    nc.vector.scalar_tensor_tensor(
                        ts[0:32, :], pp[32:64, :], -1.0,
                        sin_sb[0:32, scol], MUL, MUL)
                    nc.vector.tensor_tensor(
                        ts[32:64, :], pp[0:32, :], sin_sb[32:64, scol], MUL)
                    nc.vector.tensor_tensor(
                        qn[0:64, :], pp[0:64, :], cos_sb[0:64, scol], MUL)
                    nc.vector.tensor_tensor(
                        kt_res[0:64, st * ST:(st + 1) * ST],
                        qn[0:64, :], ts[0:64, :], ADD)
                    vt = vt_p.tile([64, ST], BF16, tag="vt")
                    nc.vector.tensor_copy(vt[:], pp[64:128, :])
                    for kb in range(4):
                        tp = fi_ps.tile([128, 66], BF16, tag="fi")
                        nc.tensor.transpose(
                            tp[:, 0:64], vt[:, kb * 128:(kb + 1) * 128],
                            ident[0:64, 0:64])
                        nc.vector.tensor_copy(
                            vp_res[:, b * NKV + sti * 4 + kb, 0:64],
                            tp[:, 0:64])
            nc.gpsimd.tensor_copy(
                kt_res[64:128, st * ST:(st + 1) * ST],
                kt_res[0:64, st * ST:(st + 1) * ST])
            if st + 1 < NST:
                load_xa(st + 1)

            # ---- attention for the 4 heads of this q-tile ----
            # full (512-col) blocks first, then the 3 partial diagonal
            # blocks with trimmed column ranges
            blocks = [(4 * sti, 0)] + [(j, 0) for j in range(4 * sti)] + \
                     [(4 * sti + 1, 128), (4 * sti + 2, 256), (4 * sti + 3, 384)]
            for h in range(HPC):
                p0 = (h % 2) * 64
                qh = (qa if h < 2 else qb)[p0:p0 + 64, :]
                cxt = cx_ps.tile([65, ST], F32, tag="cxt")
                # software-pipelined by two blocks: ctx(k-2) is emitted
                # after sc(k) so the PE has ~2 sc matmuls of work covering
                # exp(k-2)'s ACT latency
                from collections import deque
                pend = deque()

                def emit_ctx(last):
                    pj, pw0, ppsb, pidx = pend.popleft()
                    nc.tensor.matmul(
                        cxt[:, pw0:ST], vp_res[:, b * NKV + pj, 0:65],
                        ppsb[:, pw0:ST],
                        start=(pidx == 0), stop=last, skip_group_check=True)

                for idx, (j, w0) in enumerate(blocks):
                    sc = sc_ps.tile([128, ST], F32, tag="sc")
                    nc.tensor.matmul(
                        sc[:, w0:ST],
                        kt_res[p0:p0 + 64, b * S + j * 128:b * S + (j + 1) * 128],
                        qh[:, w0:ST], start=True, stop=True)
                    if j >= 4 * sti:
                        nc.vector.copy_predicated(
                            sc[:, w0:w0 + 128], mask_sb[:], neg_sb[:])
                    psb = p_p.tile([128, ST], BF16, tag="psb")
                    nc.scalar.activation(
                        psb[:, w0:ST], sc[:, w0:ST],
                        mybir.ActivationFunctionType.Exp, scale=0.125)
                    pend.append((j, w0, psb, idx))
                    if len(pend) > 1:
                        emit_ctx(False)
                while pend:
                    emit_ctx(not pend or len(pend) == 1)
                # finalize: bf16 PE transposes back to [q, d], then
                # per-partition (per-q) reciprocal of the sums column
                cxs = cx_p.tile([65, ST], BF16, tag="cxs")
                nc.vector.tensor_copy(cxs[:], cxt[:])
                for qq in range(4):
                    fi = fi_ps.tile([128, 66], BF16, tag="fi")
                    nc.tensor.transpose(fi[:], cxs[:, qq * 128:(qq + 1) * 128],
                                        ident[0:65, 0:66])
                    rv = rv_p.tile([128, 1], F32, tag="rv")
                    nc.vector.reciprocal(rv[:], fi[:, 64:65])
                    ob = o_p.tile([128, 64], F32, tag="ob")
                    nc.vector.tensor_scalar_mul(ob[:], fi[:, 0:64], rv[:])
                    nc.sync.dma_start(
                        out=out_d[st * ST + qq * 128:st * ST + (qq + 1) * 128,
                                  h * 64:(h + 1) * 64],
                        in_=ob[:])
    return nc


_NC_CACHE = None


def _host_consts():
    i = np.arange(0, D, 2, dtype=np.float64) / D          # 32 pair exponents
    freqs = 1.0 / (10000.0 ** i)                           # (32,)
    ang = np.arange(S, dtype=np.float64)[:, None] * freqs[None, :]  # (S, 32)
    cos32 = np.cos(ang).astype(np.float32).T               # (32, S)
    sin32 = np.sin(ang).astype(np.float32).T
    cosq = np.tile(cos32, (4, 1)).astype(BF16NP)           # (128, S)
    sinq = np.tile(sin32, (4, 1)).astype(BF16NP)
    kv, qq = np.meshgrid(np.arange(128), np.arange(128), indexing="ij")
    maskinv = (kv > qq).astype(np.uint8)                   # 1 = forbidden
    ident = np.eye(128, dtype=np.float32).astype(BF16NP)
    return cosq, sinq, maskinv, ident


def _deint(w):
    # (din, 64) head cols -> [evens(32) | odds(32)]
    return np.hstack([w[:, 0::2], w[:, 1::2]])


def _in_maps(x, Wq, Wk, Wv):
    x = np.asarray(x, dtype=np.float32).reshape(B * S, DIN)
    xt = np.ascontiguousarray(x.T.astype(BF16NP))
    Wq = np.asarray(Wq, dtype=np.float32)
    Wk = np.asarray(Wk, dtype=np.float32)
    Wv = np.asarray(Wv, dtype=np.float32)
    cosq, sinq, maskinv, ident = _host_consts()

    in_maps = []
    for k in range(NCORES):
        blocks = []
        for h in range(4):
            blocks.append(_deint(Wq[:, (4 * k + h) * 64:(4 * k + h + 1) * 64]))
        blocks.append(_deint(Wk[:, k * 64:(k + 1) * 64]))
        blocks.append(Wv[:, k * 64:(k + 1) * 64])
        w_all = np.hstack(blocks).astype(BF16NP)
        in_maps.append({
            "xt": xt, "w": np.ascontiguousarray(w_all),
            "cosq": cosq, "sinq": sinq, "mask": maskinv, "ident": ident,
        })
    return in_maps


def _run(in_maps, **kwargs):
    global _NC_CACHE
    if _NC_CACHE is None:
        _NC_CACHE = build_bass()
        _NC_CACHE.finalize()
    return run_bass_kernel_spmd(_NC_CACHE, in_maps, list(range(NCORES)),
                                **kwargs)


def kernel(x, Wq, Wk, Wv):
    res = _run(_in_maps(x, Wq, Wk, Wv))
    out = np.concatenate([res.results[k]["out"] for k in range(NCORES)], axis=1)
    return out.reshape(B, S, 32 * D)


# revision 62
# speedup vs baseline: 1.0587x; 1.0587x over previous
"""GQA forward (b=2, s=2048, H=32 q heads, 8 kv heads, d=64) on 8 TRN2 cores.

Sharding: core k owns query heads 4k..4k+3 and kv head k. GQA group
structure makes attention fully local per core (q heads 4k..4k+3 attend
only to kv head k). x is replicated; W columns are sharded; outputs are
column-concatenated.

Per-core kernel (Tile framework) — bf16, fully transposed dataflow:
  - Host passes x.T (pre-transposed, bf16) so no PE cycles are spent
    producing x.T tiles; W is passed bf16 with head-dim columns
    de-interleaved (evens then odds per head) so RoPE works on
    contiguous partition blocks in the transposed projection layout.
  - Projections directly in transposed layout: QKV.T[cols,s] tiles =
    W_chunk.T @ xT_chunk accumulated over 16 k-chunks (W stationary).
    Col-blocks: [q0|q1], [q2|q3], [k|v], each row-packed [e32 o32] per
    head. x.T tiles prefetched one s-tile ahead on the scalar hwdge
    queue; the sync queue carries weights/tables/outputs.
  - RoPE fused with PSUM eviction on DVE: 32-row partition-block ops,
    f32 intermediates, bf16 results straight into Q tiles / K resident
    (rows 64-127 of K duplicated via GpSimd copy so scores lhsT can
    match either Q base partition).
  - V.T rows are flipped to natural [kv, d] via 4 small PE transposes
    per s-tile into the [V|1] resident.
  - Attention in transposed layout (bf16): S.T[kv,q] = K @ Q.T per
    128-kv block, exp on ACT (scale=1/8 folded) writing P in bf16;
    causal via block skipping + per-block column trimming + triangular
    predicated masks on diagonal blocks; ctx.T[65,q] = [V|1].T @ P.T
    accumulated in PSUM (row 64 = softmax sums). The ctx matmuls are
    software-pipelined one block behind the scores matmuls so the PE
    always has work covering the ACT exp latency.
  - Finalize: bf16 PE transposes back to [q, d], per-partition
    reciprocal of the sums column, scale, DMA out f32.
"""

import numpy as np
from collections import deque
from contextlib import ExitStack

import ml_dtypes

import concourse.bass as bass
import concourse.bacc as bacc
import concourse.mybir as mybir
from concourse import tile
from concourse.bass_utils import run_bass_kernel_spmd

F32 = mybir.dt.float32
F32R = mybir.dt.float32r
BF16 = mybir.dt.bfloat16
U8 = mybir.dt.uint8
BF16NP = ml_dtypes.bfloat16
MUL = mybir.AluOpType.mult
ADD = mybir.AluOpType.add

B = 2
S = 2048
DIN = 2048
D = 64              # head dim
HPC = 4             # query heads per core
NCORES = 8
WCOLS = 384         # 3 col-blocks of 128: [q0|q1], [q2|q3], [k|v]
ST = 512            # s-tile (rows per outer step)
NST = B * S // ST   # 8 s-tiles
NCH = DIN // 128    # 16 k-chunks
NKV = S // 128      # kv tiles per batch
NEG = -30000.0      # pre-scale mask fill; exp(NEG/8) == 0 in f32


def build_bass():
    nc = bacc.Bacc(None, target_bir_lowering=False)
    xt_d = nc.declare_dram_parameter("xt", [DIN, B * S], BF16, isOutput=False)
    w_d = nc.declare_dram_parameter("w", [DIN, WCOLS], BF16, isOutput=False)
    cos_d = nc.declare_dram_parameter("cosq", [128, S], BF16, isOutput=False)
    sin_d = nc.declare_dram_parameter("sinq", [128, S], BF16, isOutput=False)
    mask_d = nc.declare_dram_parameter("mask", [128, 128], U8, isOutput=False)
    id_d = nc.declare_dram_parameter("ident", [128, 128], BF16, isOutput=False)
    out_d = nc.declare_dram_parameter("out", [B * S, HPC * D], F32, isOutput=True)

    with ExitStack() as ctx:
        tc = ctx.enter_context(tile.TileContext(nc))
        const = ctx.enter_context(tc.tile_pool(name="const", bufs=1))
        resid = ctx.enter_context(tc.tile_pool(name="resid", bufs=1))
        xa_p = ctx.enter_context(tc.tile_pool(name="xa", bufs=3))
        qt_p = ctx.enter_context(tc.tile_pool(name="qt", bufs=4))
        tmp_p = ctx.enter_context(tc.tile_pool(name="tmp", bufs=3))
        vt_p = ctx.enter_context(tc.tile_pool(name="vt", bufs=2))
        p_p = ctx.enter_context(tc.tile_pool(name="p", bufs=6))
        cx_p = ctx.enter_context(tc.tile_pool(name="cx", bufs=4))
        o_p = ctx.enter_context(tc.tile_pool(name="o", bufs=8))
        rv_p = ctx.enter_context(tc.tile_pool(name="rv", bufs=8))
        pr_ps = ctx.enter_context(tc.tile_pool(name="pr_ps", bufs=2, space="PSUM"))
        sc_ps = ctx.enter_context(tc.tile_pool(name="sc_ps", bufs=3, space="PSUM"))
        cx_ps = ctx.enter_context(tc.tile_pool(name="cx_ps", bufs=2, space="PSUM"))
        fi_ps = ctx.enter_context(tc.tile_pool(name="fi_ps", bufs=1, space="PSUM"))

        # constants — w split so the first matmul can start early (x.T
        # tiles stream in parallel on the scalar hwdge queue)
        w_dram = w_d.rearrange("(c p) n -> p c n", p=128)
        w_sb = const.tile([128, NCH, WCOLS], BF16)
        for c2 in range(0, NCH, 2):
            nc.sync.dma_start(out=w_sb[:, c2:c2 + 2, :],
                              in_=w_dram[:, c2:c2 + 2, :])
        cos_sb = const.tile([128, S], BF16)
        sin_sb = const.tile([128, S], BF16)
        # first s-tile's cos/sin lands early; later slices stream per s-tile
        nc.sync.dma_start(out=cos_sb[:, 0:ST], in_=cos_d[:, 0:ST])
        nc.sync.dma_start(out=sin_sb[:, 0:ST], in_=sin_d[:, 0:ST])
        mask_sb = const.tile([128, 128], U8)
        nc.sync.dma_start(out=mask_sb[:], in_=mask_d[:])
        ident = const.tile([128, 128], BF16)
        nc.sync.dma_start(out=ident[:], in_=id_d[:])
        neg_sb = const.tile([128, 128], F32)
        nc.vector.memset(neg_sb[:], NEG)

        # K.T resident (RoPE'd, bf16); rows 64-127 duplicate rows 0-63 so
        # the scores lhsT can match either base partition of the Q halves.
        kt_res = resid.tile([128, B * S], BF16)
        # [V | 1 | pad] kv-tiles, natural [kv, d] layout
        vp_res = resid.tile([128, B * NKV, 66], BF16)
        nc.vector.memset(vp_res[:, :, 64:65], 1.0)

        xt_dram = xt_d.rearrange("(c p) s -> p c s", p=128)

        # x.T tiles are prefetched one s-tile ahead on the scalar hwdge
        # queue (chunk-split so the first projection can start early)
        xa_tiles = {}

        def load_xa(st, step=4):
            xa = xa_p.tile([128, NCH, ST], BF16, tag="xa", name=f"xa{st}")
            for c4 in range(0, NCH, step):
                nc.scalar.dma_start(
                    out=xa[:, c4:c4 + step, :],
                    in_=xt_dram[:, c4:c4 + step, st * ST:(st + 1) * ST])
            xa_tiles[st] = xa

        load_xa(0, step=2)

        for st in range(NST):
            b, sti = divmod(st, 4)
            scol = slice(sti * ST, (sti + 1) * ST)  # within-batch position
            xa = xa_tiles.pop(st)
            if b == 0 and sti + 1 < 4:
                nxt = slice((sti + 1) * ST, (sti + 2) * ST)
                nc.scalar.dma_start(out=cos_sb[:, nxt], in_=cos_d[:, nxt])
                nc.scalar.dma_start(out=sin_sb[:, nxt], in_=sin_d[:, nxt])

            # ---- projections (transposed layout) + RoPE ----
            qa = qt_p.tile([128, ST], BF16, tag="qa")   # [q0_e q0_o q1_e q1_o]
            qb = qt_p.tile([128, ST], BF16, tag="qb")   # [q2_e q2_o q3_e q3_o]
            for cb in range(3):
                pp = pr_ps.tile([128, ST], F32, tag="pp")
                for c in range(NCH):
                    nc.tensor.matmul(
                        pp[:], w_sb[:, c, cb * 128:(cb + 1) * 128], xa[:, c, :],
                        start=(c == 0), stop=(c == NCH - 1))
                ts = tmp_p.tile([128, ST], F32, tag="ts")
                qn = tmp_p.tile([128, ST], F32, tag="qn")
                if cb < 2:
                    dst = qa if cb == 0 else qb
                    # rows per head h (0/1): [e at 64h..64h+32, o at +32..+64]
                    for hh in range(2):
                        r = 64 * hh
                        nc.vector.scalar_tensor_tensor(
                            ts[r:r + 32, :], pp[r + 32:r + 64, :], -1.0,
                            sin_sb[r:r + 32, scol], MUL, MUL)
                        nc.vector.tensor_tensor(
                            ts[r + 32:r + 64, :], pp[r:r + 32, :],
                            sin_sb[r + 32:r + 64, scol], MUL)
                    nc.vector.tensor_tensor(qn[:], pp[:], cos_sb[:, scol], MUL)
                    nc.vector.tensor_tensor(dst[:], qn[:], ts[:], ADD)
                else:
                    # rows: [k_e(32) k_o(32) | v(64)]
                    nc.vector.scalar_tensor_tensor(
                        ts[0:32, :], pp[32:64, :], -1.0,
                        sin_sb[0:32, scol], MUL, MUL)
                    nc.vector.tensor_tensor(
                        ts[32:64, :], pp[0:32, :], sin_sb[32:64, scol], MUL)
                    nc.vector.tensor_tensor(
                        qn[0:64, :], pp[0:64, :], cos_sb[0:64, scol], MUL)
                    nc.vector.tensor_tensor(
                        kt_res[0:64, st * ST:(st + 1) * ST],
                        qn[0:64, :], ts[0:64, :], ADD)
                    vt = vt_p.tile([64, ST], BF16, tag="vt")
                    nc.vector.tensor_copy(vt[:], pp[64:128, :])
                    for kb in range(4):
                        tp = fi_ps.tile([128, 66], BF16, tag="fi")
                        nc.tensor.transpose(
                            tp[:, 0:64], vt[:, kb * 128:(kb + 1) * 128],
                            ident[0:64, 0:64])
                        nc.vector.tensor_copy(
                            vp_res[:, b * NKV + sti * 4 + kb, 0:64],
                            tp[:, 0:64])
            nc.gpsimd.tensor_copy(
                kt_res[64:128, st * ST:(st + 1) * ST],
                kt_res[0:64, st * ST:(st + 1) * ST])
            if st + 1 < NST:
                load_xa(st + 1)

            # ---- attention for the 4 heads of this q-tile ----
            # full (512-col) blocks first, then the 3 partial diagonal
            # blocks with trimmed column ranges
            blocks = [(4 * sti, 0)] + [(j, 0) for j in range(4 * sti)] + \
                     [(4 * sti + 1, 128), (4 * sti + 2, 256), (4 * sti + 3, 384)]
            for h in range(HPC):
                p0 = (h % 2) * 64
                qh = (qa if h < 2 else qb)[p0:p0 + 64, :]
                cxt = cx_ps.tile([65, ST], F32, tag="cxt")
                # ctx matmuls trail the scores matmuls by one block so the
                # PE has work covering each block's ACT exp latency
                pend = deque()

                def emit_ctx(last):
                    pj, pw0, ppsb, pidx = pend.popleft()
                    nc.tensor.matmul(
                        cxt[:, pw0:ST], vp_res[:, b * NKV + pj, 0:65],
                        ppsb[:, pw0:ST],
                        start=(pidx == 0), stop=last, skip_group_check=True)

                for idx, (j, w0) in enumerate(blocks):
                    sc = sc_ps.tile([128, ST], F32, tag="sc")
                    nc.tensor.matmul(
                        sc[:, w0:ST],
                        kt_res[p0:p0 + 64, b * S + j * 128:b * S + (j + 1) * 128],
                        qh[:, w0:ST], start=True, stop=True)
                    if j >= 4 * sti:
                        nc.vector.copy_predicated(
                            sc[:, w0:w0 + 128], mask_sb[:], neg_sb[:])
                    psb = p_p.tile([128, ST], BF16, tag="psb")
                    nc.scalar.activation(
                        psb[:, w0:ST], sc[:, w0:ST],
                        mybir.ActivationFunctionType.Exp, scale=0.125)
                    pend.append((j, w0, psb, idx))
                    if len(pend) > 1:
                        emit_ctx(False)
                while pend:
                    emit_ctx(not pend or len(pend) == 1)
                # finalize: bf16 PE transposes back to [q, d], then
                # per-partition (per-q) reciprocal of the sums column
                cxs = cx_p.tile([65, ST], BF16, tag="cxs")
                nc.vector.tensor_copy(cxs[:], cxt[:])
                for qq in range(4):
                    fi = fi_ps.tile([128, 66], BF16, tag="fi")
                    nc.tensor.transpose(fi[:], cxs[:, qq * 128:(qq + 1) * 128],
                                        ident[0:65, 0:66])
                    rv = rv_p.tile([128, 1], F32, tag="rv")
                    nc.vector.reciprocal(rv[:], fi[:, 64:65])
                    ob = o_p.tile([128, 64], F32, tag="ob")
                    nc.vector.tensor_scalar_mul(ob[:], fi[:, 0:64], rv[:])
                    nc.sync.dma_start(
                        out=out_d[st * ST + qq * 128:st * ST + (qq + 1) * 128,
                                  h * 64:(h + 1) * 64],
                        in_=ob[:])
    return nc


_NC_CACHE = None


def _host_consts():
    i = np.arange(0, D, 2, dtype=np.float64) / D          # 32 pair exponents
    freqs = 1.0 / (10000.0 ** i)                           # (32,)
    ang = np.arange(S, dtype=np.float64)[:, None] * freqs[None, :]  # (S, 32)
    cos32 = np.cos(ang).astype(np.float32).T               # (32, S)
    sin32 = np.sin(ang).astype(np.float32).T
    cosq = np.tile(cos32, (4, 1)).astype(BF16NP)           # (128, S)
    sinq = np.tile(sin32, (4, 1)).astype(BF16NP)
    kv, qq = np.meshgrid(np.arange(128), np.arange(128), indexing="ij")
    maskinv = (kv > qq).astype(np.uint8)                   # 1 = forbidden
    ident = np.eye(128, dtype=np.float32).astype(BF16NP)
    return cosq, sinq, maskinv, ident


def _deint(w):
    # (din, 64) head cols -> [evens(32) | odds(32)]
    return np.hstack([w[:, 0::2], w[:, 1::2]])


def _in_maps(x, Wq, Wk, Wv):
    x = np.asarray(x, dtype=np.float32).reshape(B * S, DIN)
    xt = np.ascontiguousarray(x.T.astype(BF16NP))
    Wq = np.asarray(Wq, dtype=np.float32)
    Wk = np.asarray(Wk, dtype=np.float32)
    Wv = np.asarray(Wv, dtype=np.float32)
    cosq, sinq, maskinv, ident = _host_consts()

    in_maps = []
    for k in range(NCORES):
        blocks = []
        for h in range(4):
            blocks.append(_deint(Wq[:, (4 * k + h) * 64:(4 * k + h + 1) * 64]))
        blocks.append(_deint(Wk[:, k * 64:(k + 1) * 64]))
        blocks.append(Wv[:, k * 64:(k + 1) * 64])
        w_all = np.hstack(blocks).astype(BF16NP)
        in_maps.append({
            "xt": xt, "w": np.ascontiguousarray(w_all),
            "cosq": cosq, "sinq": sinq, "mask": maskinv, "ident": ident,
        })
    return in_maps


def _run(in_maps, **kwargs):
    global _NC_CACHE
    if _NC_CACHE is None:
        _NC_CACHE = build_bass()
        _NC_CACHE.finalize()
    return run_bass_kernel_spmd(_NC_CACHE, in_maps, list(range(NCORES)),
                                **kwargs)


def kernel(x, Wq, Wk, Wv):
    res = _run(_in_maps(x, Wq, Wk, Wv))
    out = np.concatenate([res.results[k]["out"] for k in range(NCORES)], axis=1)
    return out.reshape(B, S, 32 * D)
